# revision 9
# baseline (speedup 1.0000x reference)
"""Bahdanau attention Trainium2 kernel.

Computation (per batch b):
    q_proj = query @ Wa_w.T + Wa_b                      [H]
    k_proj = keys @ Ua_w.T + Ua_b                       [S, H]
    scores = tanh(q_proj + k_proj) @ Va_w[0] (+ Va_b)   [S]
    weights = softmax(mask ? scores : -inf)             [S]
    context = weights @ keys                            [H]

Sharding: data-parallel over batch B=32 across 8 cores (4 batches/core).

Per-core kernel strategy:
  - keys loaded once (natural [s,h] layout), PE-transposed tile-wise to
    keysT [h, s] for the big k_proj matmul (contraction over h must be on
    partitions).
  - k_proj matmul runs in float32r (full PE rate at N>=256 vs 1/4 rate
    for plain fp32), accumulated over 8 h-chunks in PSUM.
  - tanh on ACT reads PSUM directly with fused per-partition bias
    qb[g] = q_projT[g,b] + Wa_b[g] + Ua_b[g].
  - scores = Va . tanh via matmul (Va as stationary [128,1]); Va_b is a
    uniform shift of scores and cancels in softmax, so it is dropped.
  - softmax without a max pass (scores are O(1)); masking applied as a
    multiply AFTER exp (identical result: w = exp(s)*m / sum(exp(s)*m)).
  - context = wT @ keys_natural matmuls; normalization by 1/Z folded
    into the final output scale.
"""

import os

import numpy as np

import concourse.bass as bass  # noqa: F401  (AP types etc.)
import concourse.mybir as mybir
import concourse.tile as tile
from concourse import bacc
from concourse.bass_utils import run_bass_kernel_spmd
from concourse.masks import make_identity

H = 1024
B = 32
S = 2048
NCORES = 8
BL = B // NCORES  # 4 batches per core
P = 128
HC = H // P  # 8 chunks of h (and of g)
SC = 4  # s-chunks per batch
SCW = S // SC  # 512: s-chunk width
SS = SCW // P  # 4 s-subtiles per chunk

F32 = mybir.dt.float32
F32R = mybir.dt.float32r
I32 = mybir.dt.int32
TANH = mybir.ActivationFunctionType.Tanh
EXP = mybir.ActivationFunctionType.Exp


def _r(ap):
    """Reinterpret an fp32 AP as float32r for full-rate PE matmuls."""
    return ap.bitcast(F32R)


def _build(tc, keys, query, maskd, wa_w, wa_b, ua_w, ua_b, va_w, ctx_out, w_out):
    nc = tc.nc

    with (
        tc.tile_pool(name="const", bufs=1) as const_pool,
        tc.tile_pool(name="wsb", bufs=1) as wsb_pool,
        tc.tile_pool(name="tpsum", bufs=2, space="PSUM") as tpsum,
    ):
        ident = const_pool.tile([P, P], F32)
        make_identity(nc, ident[:])
        # fp32r identity for is_transpose matmuls on fp32r data; the DVE
        # copy performs the fp32->fp32r rounding the BIR verifier requires.
        ident_r = const_pool.tile([P, P], F32R)
        nc.vector.tensor_copy(ident_r[:], ident[:])

        # vaT [128, HC]: Va with g on partitions.
        vaT = const_pool.tile([P, HC], F32R)
        qT = const_pool.tile([P, HC, BL], F32R)
        qb = const_pool.tile([P, HC, BL], F32)
        bias_sum = const_pool.tile([1, H], F32)
        ones_bl = const_pool.tile([1, BL], F32)
        nc.vector.memset(ones_bl[:], 1.0)

        # uaT[p, hcc, g] = Ua_w[g, hcc*128+p]  (lhsT layout for k_proj)
        uaT = wsb_pool.tile([P, HC, H], F32R)

        with (
            tc.tile_pool(name="setup", bufs=1) as setup_pool,
            tc.tile_pool(name="qpsum", bufs=1, space="PSUM") as qpsum,
        ):
            # Biases (natural row layout) and Va.
            wab_sb = setup_pool.tile([1, H], F32, tag="wab")
            uab_sb = setup_pool.tile([1, H], F32, tag="uab")
            va_sb = setup_pool.tile([1, H], F32, tag="va")
            nc.sync.dma_start(wab_sb[:], wa_b)
            nc.sync.dma_start(uab_sb[:], ua_b)
            nc.vector.tensor_add(bias_sum[:], wab_sb[:], uab_sb[:])
            nc.sync.dma_start(va_sb[:], va_w)

            vt_ps = qpsum.tile([P, HC], F32, tag="qps")
            for i in range(HC):
                nc.tensor.transpose(
                    vt_ps[:, i : i + 1],
                    va_sb[0:1, i * P : (i + 1) * P],
                    ident[0:1, 0:1],
                )
            nc.vector.tensor_copy(vaT[:], vt_ps[:])

            # Transpose Ua -> uaT (persistent), then Wa -> waT (transient).
            for name, srcw, dstT in (("ua", ua_w, uaT), ("wa", wa_w, None)):
                nat = setup_pool.tile([P, HC, H], F32R, tag="nat")
                # SWDGE cast-DMA rounds fp32 -> fp32r in flight.
                nc.gpsimd.dma_start(nat[:], srcw.rearrange("(gc p) h -> p gc h", p=P))
                if dstT is None:
                    dstT = setup_pool.tile([P, HC, H], F32R, tag="waT")
                    waT = dstT
                for hcc in range(HC):
                    for gq in range(2):
                        ps = tpsum.tile([P, SS, P], F32R, tag="tps")
                        for j in range(SS):
                            gc = gq * SS + j
                            nc.tensor.transpose(
                                ps[:, j],
                                nat[:, gc, hcc * P : (hcc + 1) * P],
                                ident_r[:],
                            )
                        nc.vector.tensor_copy(
                            dstT[:, hcc, gq * 512 : (gq + 1) * 512],
                            ps[:].rearrange("p a f -> p (a f)"),
                        )

            # queryT [p, hcc, b] = query[b, hcc*128+p]
            q_nat = setup_pool.tile([BL, H], F32, tag="qnat")
            nc.sync.dma_start(q_nat[:], query)
            qt_ps = qpsum.tile([P, HC, BL], F32, tag="qps")
            for hcc in range(HC):
                nc.tensor.transpose(
                    qt_ps[:, hcc, :],
                    q_nat[0:BL, hcc * P : (hcc + 1) * P],
                    ident[0:BL, 0:BL],
                )
            nc.vector.tensor_copy(qT[:], qt_ps[:])

            # qb[p, gc, b] = sum_h Wa[g,h] query[b,h] + (Wa_b+Ua_b)[g]
            qp_ps = qpsum.tile([P, HC, BL], F32, tag="qps")
            for gc in range(HC):
                for hcc in range(HC):
                    nc.tensor.matmul(
                        qp_ps[:, gc, :],
                        lhsT=waT[:, hcc, gc * P : (gc + 1) * P],
                        rhs=qT[:, hcc, :],
                        start=(hcc == 0),
                        stop=False,
                    )
                nc.tensor.matmul(
                    qp_ps[:, gc, :],
                    lhsT=bias_sum[0:1, gc * P : (gc + 1) * P],
                    rhs=ones_bl[:],
                    start=False,
                    stop=True,
                )
            nc.vector.tensor_copy(qb[:], qp_ps[:])

        with (
            tc.tile_pool(name="knat", bufs=2) as knat_pool,
            tc.tile_pool(name="keysT", bufs=2) as kT_pool,
            tc.tile_pool(name="tanh", bufs=2) as th_pool,
            tc.tile_pool(name="wp", bufs=2) as wp_pool,
            tc.tile_pool(name="small", bufs=2) as small_pool,
            tc.tile_pool(name="kppsum", bufs=2, space="PSUM") as kp_psum,
            tc.tile_pool(name="spsum", bufs=2, space="PSUM") as s_psum,
            tc.tile_pool(name="cpsum", bufs=2, space="PSUM") as c_psum,
        ):
            for b in range(BL):
                wp_sb = wp_pool.tile([1, S], F32, tag="wp")
                ctx_acc = wp_pool.tile([1, H], F32, tag="ctx")
                mask_f = small_pool.tile([1, S], F32, tag="mf")
                # SWDGE DMA casts int32 -> fp32 in flight.
                nc.gpsimd.dma_start(mask_f[:], maskd[b : b + 1, :])

                for sc in range(SC):
                    ssl = slice(sc * SCW, (sc + 1) * SCW)
                    knat = knat_pool.tile([P, SS, H], F32R, tag="knat")
                    # SWDGE cast-DMA rounds fp32 -> fp32r in flight.
                    nc.gpsimd.dma_start(
                        knat[:],
                        keys[b, ssl, :].rearrange("(ss p) h -> p ss h", p=P),
                    )
                    keysT = kT_pool.tile([P, HC, SCW], F32R, tag="kT")
                    for hcc in range(HC):
                        ps = tpsum.tile([P, SS, P], F32R, tag="tps")
                        for ss in range(SS):
                            nc.tensor.transpose(
                                ps[:, ss],
                                knat[:, ss, hcc * P : (hcc + 1) * P],
                                ident_r[:],
                            )
                        nc.vector.tensor_copy(
                            keysT[:, hcc, :],
                            ps[:].rearrange("p a f -> p (a f)"),
                        )

                    tanhT = th_pool.tile([P, HC, SCW], F32R, tag="th")
                    for gc in range(HC):
                        kp = kp_psum.tile([P, SCW], F32, tag="kp")
                        for hcc in range(HC):
                            nc.tensor.matmul(
                                kp[:],
                                lhsT=uaT[:, hcc, gc * P : (gc + 1) * P],
                                rhs=keysT[:, hcc, :],
                                start=(hcc == 0),
                                stop=(hcc == HC - 1),
                            )
                        nc.scalar.activation(
                            tanhT[:, gc, :], kp[:], TANH, bias=qb[:, gc, b : b + 1]
                        )

                    sp = s_psum.tile([1, SCW], F32, tag="sp")
                    for gc in range(HC):
                        nc.tensor.matmul(
                            sp[:],
                            lhsT=vaT[:, gc : gc + 1],
                            rhs=tanhT[:, gc, :],
                            start=(gc == 0),
                            stop=(gc == HC - 1),
                        )
                    nc.scalar.activation(wp_sb[0:1, ssl], sp[:], EXP)
                    nc.vector.tensor_tensor(
                        wp_sb[0:1, ssl],
                        wp_sb[0:1, ssl],
                        mask_f[0:1, ssl],
                        mybir.AluOpType.mult,
                    )

                    # wT [s-on-partitions, 1] tiles for the context matmul.
                    # Transposes stay plain fp32 (fp32r fails the ISA check
                    # for the [1,128] shape); the copy rounds to fp32r.
                    wt_ps = tpsum.tile([P, SS, P], F32, tag="tps")
                    for ss in range(SS):
                        nc.tensor.transpose(
                            wt_ps[:, ss, 0:1],
                            wp_sb[0:1, sc * SCW + ss * P : sc * SCW + (ss + 1) * P],
                            ident[0:1, 0:1],
                        )
                    wpT = small_pool.tile([P, SS], F32R, tag="wpT")
                    nc.vector.tensor_copy(wpT[:], wt_ps[:, :, 0])

                    for h2 in range(2):
                        cp = c_psum.tile([1, 512], F32, tag="cp")
                        for ss in range(SS):
                            nc.tensor.matmul(
                                cp[:],
                                lhsT=wpT[:, ss : ss + 1],
                                rhs=knat[:, ss, h2 * 512 : (h2 + 1) * 512],
                                start=(ss == 0),
                                stop=(ss == SS - 1),
                            )
                        if sc == 0:
                            nc.vector.tensor_copy(
                                ctx_acc[0:1, h2 * 512 : (h2 + 1) * 512], cp[:]
                            )
                        else:
                            nc.vector.tensor_add(
                                ctx_acc[0:1, h2 * 512 : (h2 + 1) * 512],
                                ctx_acc[0:1, h2 * 512 : (h2 + 1) * 512],
                                cp[:],
                            )

                zsum = small_pool.tile([1, 1], F32, tag="z")
                nc.vector.reduce_sum(zsum[:], wp_sb[:], axis=mybir.AxisListType.X)
                rz = small_pool.tile([1, 1], F32, tag="rz")
                nc.vector.reciprocal(rz[:], zsum[:])
                nc.vector.tensor_scalar_mul(wp_sb[:], wp_sb[:], rz[:])
                nc.sync.dma_start(w_out[b : b + 1, :], wp_sb[:])
                nc.vector.tensor_scalar_mul(ctx_acc[:], ctx_acc[:], rz[:])
                nc.sync.dma_start(ctx_out[b : b + 1, :], ctx_acc[:])


def build_bass():
    nc = bacc.Bacc("TRN2", target_bir_lowering=False, debug=False)
    keys = nc.dram_tensor("keys", [BL, S, H], F32, kind="ExternalInput").ap()
    query = nc.dram_tensor("query", [BL, H], F32, kind="ExternalInput").ap()
    maskd = nc.dram_tensor("mask", [BL, S], I32, kind="ExternalInput").ap()
    wa_w = nc.dram_tensor("Wa_w", [H, H], F32, kind="ExternalInput").ap()
    wa_b = nc.dram_tensor("Wa_b", [1, H], F32, kind="ExternalInput").ap()
    ua_w = nc.dram_tensor("Ua_w", [H, H], F32, kind="ExternalInput").ap()
    ua_b = nc.dram_tensor("Ua_b", [1, H], F32, kind="ExternalInput").ap()
    va_w = nc.dram_tensor("Va_w", [1, H], F32, kind="ExternalInput").ap()
    ctx_out = nc.dram_tensor("context", [BL, H], F32, kind="ExternalOutput").ap()
    w_out = nc.dram_tensor("weights", [BL, S], F32, kind="ExternalOutput").ap()

    with tile.TileContext(nc) as tc:
        _build(tc, keys, query, maskd, wa_w, wa_b, ua_w, ua_b, va_w, ctx_out, w_out)
    nc.compile()
    return nc


def kernel(**inputs):
    inputs = {k: np.asarray(v) for k, v in inputs.items()}
    nc = build_bass()

    shared = {
        "Wa_w": np.ascontiguousarray(inputs["Wa_w"], dtype=np.float32),
        "Wa_b": np.ascontiguousarray(inputs["Wa_b"].reshape(1, H), dtype=np.float32),
        "Ua_w": np.ascontiguousarray(inputs["Ua_w"], dtype=np.float32),
        "Ua_b": np.ascontiguousarray(inputs["Ua_b"].reshape(1, H), dtype=np.float32),
        "Va_w": np.ascontiguousarray(inputs["Va_w"].reshape(1, H), dtype=np.float32),
    }
    in_maps = []
    for c in range(NCORES):
        sl = slice(c * BL, (c + 1) * BL)
        in_maps.append(
            {
                "query": np.ascontiguousarray(inputs["query"][sl], dtype=np.float32),
                "keys": np.ascontiguousarray(inputs["keys"][sl], dtype=np.float32),
                "mask": np.ascontiguousarray(inputs["mask"][sl], dtype=np.int32),
                **shared,
            }
        )

    trace = bool(int(os.environ.get("BAHDANAU_TRACE", "0")))
    res = run_bass_kernel_spmd(
        nc, in_maps, core_ids=list(range(NCORES)), trace=trace
    )
    if trace and res.exec_time_ns is not None:
        print(f"HW exec time: {res.exec_time_ns} ns")
        if res.instructions_and_trace is not None:
            print(f"trace: {res.instructions_and_trace[1]}")

    context = np.concatenate([r["context"] for r in res.results], axis=0)
    weights = np.concatenate([r["weights"] for r in res.results], axis=0)
    return context, weights


if __name__ == "__main__":
    rng = np.random.default_rng(0)
    scale = 1.0 / np.sqrt(H)
    ins = {
        "query": rng.standard_normal((B, H), dtype=np.float32),
        "keys": rng.standard_normal((B, S, H), dtype=np.float32),
        "mask": rng.integers(0, 2, size=(B, S)).astype(np.int32),
        "Wa_w": rng.uniform(-scale, scale, (H, H)).astype(np.float32),
        "Wa_b": rng.uniform(-scale, scale, (H,)).astype(np.float32),
        "Ua_w": rng.uniform(-scale, scale, (H, H)).astype(np.float32),
        "Ua_b": rng.uniform(-scale, scale, (H,)).astype(np.float32),
        "Va_w": rng.uniform(-scale, scale, (1, H)).astype(np.float32),
        "Va_b": rng.uniform(-scale, scale, (1,)).astype(np.float32),
    }
    ctx, wts = kernel(**ins)
    print("context", ctx.shape, ctx.dtype, "weights", wts.shape, wts.dtype)


# revision 10
# speedup vs baseline: 10090.7813x; 10090.7813x over previous
"""Bahdanau attention Trainium2 kernel.

Computation (per batch b):
    q_proj = query @ Wa_w.T + Wa_b                      [H]
    k_proj = keys @ Ua_w.T + Ua_b                       [S, H]
    scores = tanh(q_proj + k_proj) @ Va_w[0] (+ Va_b)   [S]
    weights = softmax(mask ? scores : -inf)             [S]
    context = weights @ keys                            [H]

Sharding: data-parallel over batch B=32 across 8 cores (4 batches/core).

Per-core kernel strategy:
  - keys loaded once (natural [s,h] layout), PE-transposed tile-wise to
    keysT [h, s] for the big k_proj matmul (contraction over h must be on
    partitions).
  - k_proj matmul runs in float32r (full PE rate at N>=256 vs 1/4 rate
    for plain fp32), accumulated over 8 h-chunks in PSUM.
  - tanh on ACT reads PSUM directly with fused per-partition bias
    qb[g] = q_projT[g,b] + Wa_b[g] + Ua_b[g].
  - scores = Va . tanh via matmul (Va as stationary [128,1]); Va_b is a
    uniform shift of scores and cancels in softmax, so it is dropped.
  - softmax without a max pass (scores are O(1)); masking applied as a
    multiply AFTER exp (identical result: w = exp(s)*m / sum(exp(s)*m)).
  - context = wT @ keys_natural matmuls; normalization by 1/Z folded
    into the final output scale.
"""

import os

import numpy as np

import concourse.bass as bass  # noqa: F401  (AP types etc.)
import concourse.mybir as mybir
import concourse.tile as tile
from concourse import bacc
from concourse.bass_utils import run_bass_kernel_spmd
from concourse.masks import make_identity

H = 1024
B = 32
S = 2048
NCORES = 8
BL = B // NCORES  # 4 batches per core
P = 128
HC = H // P  # 8 chunks of h (and of g)
SC = 4  # s-chunks per batch
SCW = S // SC  # 512: s-chunk width
SS = SCW // P  # 4 s-subtiles per chunk

F32 = mybir.dt.float32
F32R = mybir.dt.float32r
I32 = mybir.dt.int32
TANH = mybir.ActivationFunctionType.Tanh
EXP = mybir.ActivationFunctionType.Exp


def _r(ap):
    """Reinterpret an fp32 AP as float32r for full-rate PE matmuls."""
    return ap.bitcast(F32R)


def _build(tc, keys, query, maskd, wa_w, wa_b, ua_w, ua_b, va_w, ctx_out, w_out, reps=1):
    nc = tc.nc

    with (
        tc.tile_pool(name="const", bufs=1) as const_pool,
        tc.tile_pool(name="wsb", bufs=1) as wsb_pool,
        tc.tile_pool(name="tpsum", bufs=2, space="PSUM") as tpsum,
    ):
        ident = const_pool.tile([P, P], F32)
        make_identity(nc, ident[:])
        # fp32r identity for is_transpose matmuls on fp32r data; the DVE
        # copy performs the fp32->fp32r rounding the BIR verifier requires.
        ident_r = const_pool.tile([P, P], F32R)
        nc.vector.tensor_copy(ident_r[:], ident[:])

        # vaT [128, HC]: Va with g on partitions.
        vaT = const_pool.tile([P, HC], F32R)
        qT = const_pool.tile([P, HC, BL], F32R)
        qb = const_pool.tile([P, HC, BL], F32)
        bias_sum = const_pool.tile([1, H], F32)
        ones_bl = const_pool.tile([1, BL], F32)
        nc.vector.memset(ones_bl[:], 1.0)

        # uaT[p, hcc, g] = Ua_w[g, hcc*128+p]  (lhsT layout for k_proj)
        uaT = wsb_pool.tile([P, HC, H], F32R)

        with (
            tc.tile_pool(name="setup", bufs=1) as setup_pool,
            tc.tile_pool(name="qpsum", bufs=1, space="PSUM") as qpsum,
        ):
            # Biases (natural row layout) and Va.
            wab_sb = setup_pool.tile([1, H], F32, tag="wab")
            uab_sb = setup_pool.tile([1, H], F32, tag="uab")
            va_sb = setup_pool.tile([1, H], F32, tag="va")
            nc.sync.dma_start(wab_sb[:], wa_b)
            nc.sync.dma_start(uab_sb[:], ua_b)
            nc.vector.tensor_add(bias_sum[:], wab_sb[:], uab_sb[:])
            nc.sync.dma_start(va_sb[:], va_w)

            vt_ps = qpsum.tile([P, HC], F32, tag="qps")
            for i in range(HC):
                nc.tensor.transpose(
                    vt_ps[:, i : i + 1],
                    va_sb[0:1, i * P : (i + 1) * P],
                    ident[0:1, 0:1],
                )
            nc.vector.tensor_copy(vaT[:], vt_ps[:])

            # Transpose Ua -> uaT (persistent), then Wa -> waT (transient).
            for name, srcw, dstT in (("ua", ua_w, uaT), ("wa", wa_w, None)):
                nat = setup_pool.tile([P, HC, H], F32R, tag="nat")
                # SWDGE cast-DMA rounds fp32 -> fp32r in flight.
                nc.gpsimd.dma_start(nat[:], srcw.rearrange("(gc p) h -> p gc h", p=P))
                if dstT is None:
                    dstT = setup_pool.tile([P, HC, H], F32R, tag="waT")
                    waT = dstT
                for hcc in range(HC):
                    for gq in range(2):
                        ps = tpsum.tile([P, SS, P], F32R, tag="tps")
                        for j in range(SS):
                            gc = gq * SS + j
                            nc.tensor.transpose(
                                ps[:, j],
                                nat[:, gc, hcc * P : (hcc + 1) * P],
                                ident_r[:],
                            )
                        nc.vector.tensor_copy(
                            dstT[:, hcc, gq * 512 : (gq + 1) * 512],
                            ps[:].rearrange("p a f -> p (a f)"),
                        )

            # queryT [p, hcc, b] = query[b, hcc*128+p]
            q_nat = setup_pool.tile([BL, H], F32, tag="qnat")
            nc.sync.dma_start(q_nat[:], query)
            qt_ps = qpsum.tile([P, HC, BL], F32, tag="qps")
            for hcc in range(HC):
                nc.tensor.transpose(
                    qt_ps[:, hcc, :],
                    q_nat[0:BL, hcc * P : (hcc + 1) * P],
                    ident[0:BL, 0:BL],
                )
            nc.vector.tensor_copy(qT[:], qt_ps[:])

            # qb[p, gc, b] = sum_h Wa[g,h] query[b,h] + (Wa_b+Ua_b)[g]
            qp_ps = qpsum.tile([P, HC, BL], F32, tag="qps")
            for gc in range(HC):
                for hcc in range(HC):
                    nc.tensor.matmul(
                        qp_ps[:, gc, :],
                        lhsT=waT[:, hcc, gc * P : (gc + 1) * P],
                        rhs=qT[:, hcc, :],
                        start=(hcc == 0),
                        stop=False,
                    )
                nc.tensor.matmul(
                    qp_ps[:, gc, :],
                    lhsT=bias_sum[0:1, gc * P : (gc + 1) * P],
                    rhs=ones_bl[:],
                    start=False,
                    stop=True,
                )
            nc.vector.tensor_copy(qb[:], qp_ps[:])

        with (
            tc.tile_pool(name="knat", bufs=2) as knat_pool,
            tc.tile_pool(name="keysT", bufs=2) as kT_pool,
            tc.tile_pool(name="tanh", bufs=2) as th_pool,
            tc.tile_pool(name="wp", bufs=2) as wp_pool,
            tc.tile_pool(name="small", bufs=2) as small_pool,
            tc.tile_pool(name="kppsum", bufs=2, space="PSUM") as kp_psum,
            tc.tile_pool(name="spsum", bufs=2, space="PSUM") as s_psum,
            tc.tile_pool(name="cpsum", bufs=2, space="PSUM") as c_psum,
        ):
            for _rep in range(reps):
              for b in range(BL):
                wp_sb = wp_pool.tile([1, S], F32, tag="wp")
                ctx_acc = wp_pool.tile([1, H], F32, tag="ctx")
                mask_f = small_pool.tile([1, S], F32, tag="mf")
                # SWDGE DMA casts int32 -> fp32 in flight.
                nc.gpsimd.dma_start(mask_f[:], maskd[b : b + 1, :])

                for sc in range(SC):
                    ssl = slice(sc * SCW, (sc + 1) * SCW)
                    knat = knat_pool.tile([P, SS, H], F32R, tag="knat")
                    # SWDGE cast-DMA rounds fp32 -> fp32r in flight.
                    nc.gpsimd.dma_start(
                        knat[:],
                        keys[b, ssl, :].rearrange("(ss p) h -> p ss h", p=P),
                    )
                    keysT = kT_pool.tile([P, HC, SCW], F32R, tag="kT")
                    for hcc in range(HC):
                        ps = tpsum.tile([P, SS, P], F32R, tag="tps")
                        for ss in range(SS):
                            nc.tensor.transpose(
                                ps[:, ss],
                                knat[:, ss, hcc * P : (hcc + 1) * P],
                                ident_r[:],
                            )
                        nc.vector.tensor_copy(
                            keysT[:, hcc, :],
                            ps[:].rearrange("p a f -> p (a f)"),
                        )

                    tanhT = th_pool.tile([P, HC, SCW], F32R, tag="th")
                    for gc in range(HC):
                        kp = kp_psum.tile([P, SCW], F32, tag="kp")
                        for hcc in range(HC):
                            nc.tensor.matmul(
                                kp[:],
                                lhsT=uaT[:, hcc, gc * P : (gc + 1) * P],
                                rhs=keysT[:, hcc, :],
                                start=(hcc == 0),
                                stop=(hcc == HC - 1),
                            )
                        nc.scalar.activation(
                            tanhT[:, gc, :], kp[:], TANH, bias=qb[:, gc, b : b + 1]
                        )

                    sp = s_psum.tile([1, SCW], F32, tag="sp")
                    for gc in range(HC):
                        nc.tensor.matmul(
                            sp[:],
                            lhsT=vaT[:, gc : gc + 1],
                            rhs=tanhT[:, gc, :],
                            start=(gc == 0),
                            stop=(gc == HC - 1),
                        )
                    nc.scalar.activation(wp_sb[0:1, ssl], sp[:], EXP)
                    nc.vector.tensor_tensor(
                        wp_sb[0:1, ssl],
                        wp_sb[0:1, ssl],
                        mask_f[0:1, ssl],
                        mybir.AluOpType.mult,
                    )

                    # wT [s-on-partitions, 1] tiles for the context matmul.
                    # Transposes stay plain fp32 (fp32r fails the ISA check
                    # for the [1,128] shape); the copy rounds to fp32r.
                    wt_ps = tpsum.tile([P, SS, P], F32, tag="tps")
                    for ss in range(SS):
                        nc.tensor.transpose(
                            wt_ps[:, ss, 0:1],
                            wp_sb[0:1, sc * SCW + ss * P : sc * SCW + (ss + 1) * P],
                            ident[0:1, 0:1],
                        )
                    wpT = small_pool.tile([P, SS], F32R, tag="wpT")
                    nc.vector.tensor_copy(wpT[:], wt_ps[:, :, 0])

                    for h2 in range(2):
                        cp = c_psum.tile([1, 512], F32, tag="cp")
                        for ss in range(SS):
                            nc.tensor.matmul(
                                cp[:],
                                lhsT=wpT[:, ss : ss + 1],
                                rhs=knat[:, ss, h2 * 512 : (h2 + 1) * 512],
                                start=(ss == 0),
                                stop=(ss == SS - 1),
                            )
                        if sc == 0:
                            nc.vector.tensor_copy(
                                ctx_acc[0:1, h2 * 512 : (h2 + 1) * 512], cp[:]
                            )
                        else:
                            nc.vector.tensor_add(
                                ctx_acc[0:1, h2 * 512 : (h2 + 1) * 512],
                                ctx_acc[0:1, h2 * 512 : (h2 + 1) * 512],
                                cp[:],
                            )

                zsum = small_pool.tile([1, 1], F32, tag="z")
                nc.vector.reduce_sum(zsum[:], wp_sb[:], axis=mybir.AxisListType.X)
                rz = small_pool.tile([1, 1], F32, tag="rz")
                nc.vector.reciprocal(rz[:], zsum[:])
                nc.vector.tensor_scalar_mul(wp_sb[:], wp_sb[:], rz[:])
                nc.sync.dma_start(w_out[b : b + 1, :], wp_sb[:])
                nc.vector.tensor_scalar_mul(ctx_acc[:], ctx_acc[:], rz[:])
                nc.sync.dma_start(ctx_out[b : b + 1, :], ctx_acc[:])


def build_bass(reps=1):
    nc = bacc.Bacc("TRN2", target_bir_lowering=False, debug=False)
    keys = nc.dram_tensor("keys", [BL, S, H], F32, kind="ExternalInput").ap()
    query = nc.dram_tensor("query", [BL, H], F32, kind="ExternalInput").ap()
    maskd = nc.dram_tensor("mask", [BL, S], I32, kind="ExternalInput").ap()
    wa_w = nc.dram_tensor("Wa_w", [H, H], F32, kind="ExternalInput").ap()
    wa_b = nc.dram_tensor("Wa_b", [1, H], F32, kind="ExternalInput").ap()
    ua_w = nc.dram_tensor("Ua_w", [H, H], F32, kind="ExternalInput").ap()
    ua_b = nc.dram_tensor("Ua_b", [1, H], F32, kind="ExternalInput").ap()
    va_w = nc.dram_tensor("Va_w", [1, H], F32, kind="ExternalInput").ap()
    ctx_out = nc.dram_tensor("context", [BL, H], F32, kind="ExternalOutput").ap()
    w_out = nc.dram_tensor("weights", [BL, S], F32, kind="ExternalOutput").ap()

    with tile.TileContext(nc) as tc:
        _build(
            tc, keys, query, maskd, wa_w, wa_b, ua_w, ua_b, va_w, ctx_out, w_out,
            reps=reps,
        )
    nc.compile()
    return nc


def kernel(**inputs):
    inputs = {k: np.asarray(v) for k, v in inputs.items()}
    nc = build_bass()

    shared = {
        "Wa_w": np.ascontiguousarray(inputs["Wa_w"], dtype=np.float32),
        "Wa_b": np.ascontiguousarray(inputs["Wa_b"].reshape(1, H), dtype=np.float32),
        "Ua_w": np.ascontiguousarray(inputs["Ua_w"], dtype=np.float32),
        "Ua_b": np.ascontiguousarray(inputs["Ua_b"].reshape(1, H), dtype=np.float32),
        "Va_w": np.ascontiguousarray(inputs["Va_w"].reshape(1, H), dtype=np.float32),
    }
    in_maps = []
    for c in range(NCORES):
        sl = slice(c * BL, (c + 1) * BL)
        in_maps.append(
            {
                "query": np.ascontiguousarray(inputs["query"][sl], dtype=np.float32),
                "keys": np.ascontiguousarray(inputs["keys"][sl], dtype=np.float32),
                "mask": np.ascontiguousarray(inputs["mask"][sl], dtype=np.int32),
                **shared,
            }
        )

    trace = bool(int(os.environ.get("BAHDANAU_TRACE", "0")))
    res = run_bass_kernel_spmd(
        nc, in_maps, core_ids=list(range(NCORES)), trace=trace
    )
    if trace and res.exec_time_ns is not None:
        print(f"HW exec time: {res.exec_time_ns} ns")
        if res.instructions_and_trace is not None:
            print(f"trace: {res.instructions_and_trace[1]}")

    context = np.concatenate([r["context"] for r in res.results], axis=0)
    weights = np.concatenate([r["weights"] for r in res.results], axis=0)
    return context, weights


if __name__ == "__main__":
    rng = np.random.default_rng(0)
    scale = 1.0 / np.sqrt(H)
    ins = {
        "query": rng.standard_normal((B, H), dtype=np.float32),
        "keys": rng.standard_normal((B, S, H), dtype=np.float32),
        "mask": rng.integers(0, 2, size=(B, S)).astype(np.int32),
        "Wa_w": rng.uniform(-scale, scale, (H, H)).astype(np.float32),
        "Wa_b": rng.uniform(-scale, scale, (H,)).astype(np.float32),
        "Ua_w": rng.uniform(-scale, scale, (H, H)).astype(np.float32),
        "Ua_b": rng.uniform(-scale, scale, (H,)).astype(np.float32),
        "Va_w": rng.uniform(-scale, scale, (1, H)).astype(np.float32),
        "Va_b": rng.uniform(-scale, scale, (1,)).astype(np.float32),
    }
    ctx, wts = kernel(**ins)
    print("context", ctx.shape, ctx.dtype, "weights", wts.shape, wts.dtype)


# revision 12
# speedup vs baseline: 12057.8349x; 1.1949x over previous
"""Bahdanau attention Trainium2 kernel.

Computation (per batch b):
    q_proj = query @ Wa_w.T + Wa_b                      [H]
    k_proj = keys @ Ua_w.T + Ua_b                       [S, H]
    scores = tanh(q_proj + k_proj) @ Va_w[0] (+ Va_b)   [S]
    weights = softmax(mask ? scores : -inf)             [S]
    context = weights @ keys                            [H]

Sharding: data-parallel over batch B=32 across 8 cores (4 batches/core).

Per-core kernel strategy:
  - keys loaded once (natural [s,h] layout), PE-transposed tile-wise to
    keysT [h, s] for the big k_proj matmul (contraction over h must be on
    partitions).
  - k_proj matmul runs in float32r (full PE rate at N>=256 vs 1/4 rate
    for plain fp32), accumulated over 8 h-chunks in PSUM.
  - tanh on ACT reads PSUM directly with fused per-partition bias
    qb[g] = q_projT[g,b] + Wa_b[g] + Ua_b[g].
  - scores = Va . tanh via matmul (Va as stationary [128,1]); Va_b is a
    uniform shift of scores and cancels in softmax, so it is dropped.
  - softmax without a max pass (scores are O(1)); masking applied as a
    multiply AFTER exp (identical result: w = exp(s)*m / sum(exp(s)*m)).
  - context = wT @ keys_natural matmuls; normalization by 1/Z folded
    into the final output scale.
"""

import os

import numpy as np

import concourse.bass as bass  # noqa: F401  (AP types etc.)
import concourse.mybir as mybir
import concourse.tile as tile
from concourse import bacc
from concourse.bass_utils import run_bass_kernel_spmd
from concourse.masks import make_identity

H = 1024
B = 32
S = 2048
NCORES = 8
BL = B // NCORES  # 4 batches per core
P = 128
HC = H // P  # 8 chunks of h (and of g)
SC = 4  # s-chunks per batch
SCW = S // SC  # 512: s-chunk width
SS = SCW // P  # 4 s-subtiles per chunk

F32 = mybir.dt.float32
F32R = mybir.dt.float32r
I32 = mybir.dt.int32
TANH = mybir.ActivationFunctionType.Tanh
EXP = mybir.ActivationFunctionType.Exp


def _r(ap):
    """Reinterpret an fp32 AP as float32r for full-rate PE matmuls."""
    return ap.bitcast(F32R)


def _build(tc, keys, query, maskd, wa_w, wa_b, ua_w, ua_b, va_w, ctx_out, w_out, reps=1):
    nc = tc.nc

    with (
        tc.tile_pool(name="const", bufs=1) as const_pool,
        tc.tile_pool(name="wsb", bufs=1) as wsb_pool,
        tc.tile_pool(name="tpsum", bufs=2, space="PSUM") as tpsum,
    ):
        ident = const_pool.tile([P, P], F32)
        make_identity(nc, ident[:])
        # fp32r identity for is_transpose matmuls on fp32r data; the DVE
        # copy performs the fp32->fp32r rounding the BIR verifier requires.
        ident_r = const_pool.tile([P, P], F32R)
        nc.vector.tensor_copy(ident_r[:], ident[:])

        # vaT [128, HC]: Va with g on partitions.
        vaT = const_pool.tile([P, HC], F32R)
        qT = const_pool.tile([P, HC, BL], F32R)
        qb = const_pool.tile([P, HC, BL], F32)
        bias_sum = const_pool.tile([1, H], F32)
        ones_bl = const_pool.tile([1, BL], F32)
        nc.vector.memset(ones_bl[:], 1.0)

        # uaT[p, hcc, g] = Ua_w[g, hcc*128+p]  (lhsT layout for k_proj)
        uaT = wsb_pool.tile([P, HC, H], F32R)

        with (
            tc.tile_pool(name="setup", bufs=1) as setup_pool,
            tc.tile_pool(name="qpsum", bufs=1, space="PSUM") as qpsum,
        ):
            # Biases (natural row layout) and Va.
            wab_sb = setup_pool.tile([1, H], F32, tag="wab")
            uab_sb = setup_pool.tile([1, H], F32, tag="uab")
            va_sb = setup_pool.tile([1, H], F32, tag="va")
            nc.sync.dma_start(wab_sb[:], wa_b)
            nc.sync.dma_start(uab_sb[:], ua_b)
            nc.vector.tensor_add(bias_sum[:], wab_sb[:], uab_sb[:])
            nc.sync.dma_start(va_sb[:], va_w)

            vt_ps = qpsum.tile([P, HC], F32, tag="qps")
            for i in range(HC):
                nc.tensor.transpose(
                    vt_ps[:, i : i + 1],
                    va_sb[0:1, i * P : (i + 1) * P],
                    ident[0:1, 0:1],
                )
            nc.vector.tensor_copy(vaT[:], vt_ps[:])

            # Transpose Ua -> uaT (persistent), then Wa -> waT (transient).
            for name, srcw, dstT in (("ua", ua_w, uaT), ("wa", wa_w, None)):
                nat = setup_pool.tile([P, HC, H], F32R, tag="nat")
                # SWDGE cast-DMA rounds fp32 -> fp32r in flight.
                nc.gpsimd.dma_start(nat[:], srcw.rearrange("(gc p) h -> p gc h", p=P))
                if dstT is None:
                    dstT = setup_pool.tile([P, HC, H], F32R, tag="waT")
                    waT = dstT
                for hcc in range(HC):
                    for gq in range(2):
                        ps = tpsum.tile([P, SS, P], F32R, tag="tps")
                        for j in range(SS):
                            gc = gq * SS + j
                            nc.tensor.transpose(
                                ps[:, j],
                                nat[:, gc, hcc * P : (hcc + 1) * P],
                                ident_r[:],
                            )
                        nc.vector.tensor_copy(
                            dstT[:, hcc, gq * 512 : (gq + 1) * 512],
                            ps[:].rearrange("p a f -> p (a f)"),
                        )

            # queryT [p, hcc, b] = query[b, hcc*128+p]
            q_nat = setup_pool.tile([BL, H], F32, tag="qnat")
            nc.sync.dma_start(q_nat[:], query)
            qt_ps = qpsum.tile([P, HC, BL], F32, tag="qps")
            for hcc in range(HC):
                nc.tensor.transpose(
                    qt_ps[:, hcc, :],
                    q_nat[0:BL, hcc * P : (hcc + 1) * P],
                    ident[0:BL, 0:BL],
                )
            nc.vector.tensor_copy(qT[:], qt_ps[:])

            # qb[p, gc, b] = sum_h Wa[g,h] query[b,h] + (Wa_b+Ua_b)[g]
            qp_ps = qpsum.tile([P, HC, BL], F32, tag="qps")
            for gc in range(HC):
                for hcc in range(HC):
                    nc.tensor.matmul(
                        qp_ps[:, gc, :],
                        lhsT=waT[:, hcc, gc * P : (gc + 1) * P],
                        rhs=qT[:, hcc, :],
                        start=(hcc == 0),
                        stop=False,
                    )
                nc.tensor.matmul(
                    qp_ps[:, gc, :],
                    lhsT=bias_sum[0:1, gc * P : (gc + 1) * P],
                    rhs=ones_bl[:],
                    start=False,
                    stop=True,
                )
            nc.vector.tensor_copy(qb[:], qp_ps[:])

        with (
            tc.tile_pool(name="knat", bufs=3) as knat_pool,
            tc.tile_pool(name="keysT", bufs=2) as kT_pool,
            tc.tile_pool(name="tanh", bufs=2) as th_pool,
            tc.tile_pool(name="wp", bufs=2) as wp_pool,
            tc.tile_pool(name="small", bufs=2) as small_pool,
            tc.tile_pool(name="kppsum", bufs=2, space="PSUM") as kp_psum,
            tc.tile_pool(name="spsum", bufs=2, space="PSUM") as s_psum,
            tc.tile_pool(name="cpsum", bufs=2, space="PSUM") as c_psum,
        ):
            # Software pipeline over chunks: per emission step i the PE sees
            #   trans(i), scores(i-1)+wpT(i-1), kproj(i), context(i-1)
            # so the big kproj(i) covers the ACT/DVE softmax tail of chunk
            # i-1 and the PE never idles on the exp/mask/wpT ping-pong.
            chunks = [
                (rep, b, sc)
                for rep in range(reps)
                for b in range(BL)
                for sc in range(SC)
            ]
            bstate = {}
            cstate = {}

            def stage_a(i):
                rep, b, sc = chunks[i]
                if sc == 0:
                    wp_sb = wp_pool.tile([1, S], F32, tag="wp")
                    ctx_acc = wp_pool.tile([1, H], F32, tag="ctx")
                    mask_f = small_pool.tile([1, S], F32, tag="mf")
                    bs = {"wp_sb": wp_sb, "ctx_acc": ctx_acc, "mask_f": mask_f}
                    # SWDGE DMA casts int32 -> fp32 in flight.
                    nc.gpsimd.dma_start(bs["mask_f"][:], maskd[b : b + 1, :])
                    bstate[(rep, b)] = bs
                ssl = slice(sc * SCW, (sc + 1) * SCW)
                knat = knat_pool.tile([P, SS, H], F32R, tag="knat")
                # SWDGE cast-DMA rounds fp32 -> fp32r in flight.
                nc.gpsimd.dma_start(
                    knat[:],
                    keys[b, ssl, :].rearrange("(ss p) h -> p ss h", p=P),
                )
                keysT = kT_pool.tile([P, HC, SCW], F32R, tag="kT")
                for hcc in range(HC):
                    ps = tpsum.tile([P, SS, P], F32R, tag="tps")
                    for ss in range(SS):
                        nc.tensor.transpose(
                            ps[:, ss],
                            knat[:, ss, hcc * P : (hcc + 1) * P],
                            ident_r[:],
                        )
                    nc.vector.tensor_copy(
                        keysT[:, hcc, :],
                        ps[:].rearrange("p a f -> p (a f)"),
                    )
                cstate[i] = {"knat": knat, "keysT": keysT, "ssl": ssl}

            def stage_b(i):
                rep, b, sc = chunks[i]
                cs = cstate[i]
                tanhT = th_pool.tile([P, HC, SCW], F32R, tag="th")
                for gc in range(HC):
                    kp = kp_psum.tile([P, SCW], F32, tag="kp")
                    for hcc in range(HC):
                        nc.tensor.matmul(
                            kp[:],
                            lhsT=uaT[:, hcc, gc * P : (gc + 1) * P],
                            rhs=cs["keysT"][:, hcc, :],
                            start=(hcc == 0),
                            stop=(hcc == HC - 1),
                        )
                    nc.scalar.activation(
                        tanhT[:, gc, :], kp[:], TANH, bias=qb[:, gc, b : b + 1]
                    )
                cs["tanhT"] = tanhT

            def stage_c(i):
                rep, b, sc = chunks[i]
                cs = cstate[i]
                bs = bstate[(rep, b)]
                wp_sb = bs["wp_sb"]
                ssl = cs["ssl"]
                sp = s_psum.tile([1, SCW], F32, tag="sp")
                for gc in range(HC):
                    nc.tensor.matmul(
                        sp[:],
                        lhsT=vaT[:, gc : gc + 1],
                        rhs=cs["tanhT"][:, gc, :],
                        start=(gc == 0),
                        stop=(gc == HC - 1),
                    )
                nc.scalar.activation(wp_sb[0:1, ssl], sp[:], EXP)
                nc.vector.tensor_tensor(
                    wp_sb[0:1, ssl],
                    wp_sb[0:1, ssl],
                    bs["mask_f"][0:1, ssl],
                    mybir.AluOpType.mult,
                )
                # wT [s-on-partitions, 1] tiles for the context matmul.
                # Transposes stay plain fp32 (fp32r fails the ISA check for
                # the [1,128] shape); the copy rounds to fp32r.
                wt_ps = tpsum.tile([P, SS, P], F32, tag="tps")
                for ss in range(SS):
                    nc.tensor.transpose(
                        wt_ps[:, ss, 0:1],
                        wp_sb[0:1, sc * SCW + ss * P : sc * SCW + (ss + 1) * P],
                        ident[0:1, 0:1],
                    )
                wpT = small_pool.tile([P, SS], F32R, tag="wpT")
                nc.vector.tensor_copy(wpT[:], wt_ps[:, :, 0])
                cs["wpT"] = wpT

            def stage_d(i):
                rep, b, sc = chunks[i]
                cs = cstate.pop(i)
                bs = bstate[(rep, b)]
                ctx_acc = bs["ctx_acc"]
                for h2 in range(2):
                    cp = c_psum.tile([1, 512], F32, tag="cp")
                    for ss in range(SS):
                        nc.tensor.matmul(
                            cp[:],
                            lhsT=cs["wpT"][:, ss : ss + 1],
                            rhs=cs["knat"][:, ss, h2 * 512 : (h2 + 1) * 512],
                            start=(ss == 0),
                            stop=(ss == SS - 1),
                        )
                    if sc == 0:
                        nc.vector.tensor_copy(
                            ctx_acc[0:1, h2 * 512 : (h2 + 1) * 512], cp[:]
                        )
                    else:
                        nc.vector.tensor_add(
                            ctx_acc[0:1, h2 * 512 : (h2 + 1) * 512],
                            ctx_acc[0:1, h2 * 512 : (h2 + 1) * 512],
                            cp[:],
                        )
                if sc == SC - 1:
                    bs = bstate.pop((rep, b))
                    wp_sb = bs["wp_sb"]
                    zsum = small_pool.tile([1, 1], F32, tag="z")
                    nc.vector.reduce_sum(
                        zsum[:], wp_sb[:], axis=mybir.AxisListType.X
                    )
                    rz = small_pool.tile([1, 1], F32, tag="rz")
                    nc.vector.reciprocal(rz[:], zsum[:])
                    nc.vector.tensor_scalar_mul(wp_sb[:], wp_sb[:], rz[:])
                    nc.sync.dma_start(w_out[b : b + 1, :], wp_sb[:])
                    nc.vector.tensor_scalar_mul(ctx_acc[:], ctx_acc[:], rz[:])
                    nc.sync.dma_start(ctx_out[b : b + 1, :], ctx_acc[:])

            n = len(chunks)
            for i in range(n + 1):
                if i < n:
                    stage_a(i)
                if i >= 1:
                    stage_c(i - 1)
                if i < n:
                    stage_b(i)
                if i >= 1:
                    stage_d(i - 1)


def build_bass(reps=1):
    nc = bacc.Bacc("TRN2", target_bir_lowering=False, debug=False)
    keys = nc.dram_tensor("keys", [BL, S, H], F32, kind="ExternalInput").ap()
    query = nc.dram_tensor("query", [BL, H], F32, kind="ExternalInput").ap()
    maskd = nc.dram_tensor("mask", [BL, S], I32, kind="ExternalInput").ap()
    wa_w = nc.dram_tensor("Wa_w", [H, H], F32, kind="ExternalInput").ap()
    wa_b = nc.dram_tensor("Wa_b", [1, H], F32, kind="ExternalInput").ap()
    ua_w = nc.dram_tensor("Ua_w", [H, H], F32, kind="ExternalInput").ap()
    ua_b = nc.dram_tensor("Ua_b", [1, H], F32, kind="ExternalInput").ap()
    va_w = nc.dram_tensor("Va_w", [1, H], F32, kind="ExternalInput").ap()
    ctx_out = nc.dram_tensor("context", [BL, H], F32, kind="ExternalOutput").ap()
    w_out = nc.dram_tensor("weights", [BL, S], F32, kind="ExternalOutput").ap()

    with tile.TileContext(nc) as tc:
        _build(
            tc, keys, query, maskd, wa_w, wa_b, ua_w, ua_b, va_w, ctx_out, w_out,
            reps=reps,
        )
    nc.compile()
    return nc


def kernel(**inputs):
    inputs = {k: np.asarray(v) for k, v in inputs.items()}
    nc = build_bass()

    shared = {
        "Wa_w": np.ascontiguousarray(inputs["Wa_w"], dtype=np.float32),
        "Wa_b": np.ascontiguousarray(inputs["Wa_b"].reshape(1, H), dtype=np.float32),
        "Ua_w": np.ascontiguousarray(inputs["Ua_w"], dtype=np.float32),
        "Ua_b": np.ascontiguousarray(inputs["Ua_b"].reshape(1, H), dtype=np.float32),
        "Va_w": np.ascontiguousarray(inputs["Va_w"].reshape(1, H), dtype=np.float32),
    }
    in_maps = []
    for c in range(NCORES):
        sl = slice(c * BL, (c + 1) * BL)
        in_maps.append(
            {
                "query": np.ascontiguousarray(inputs["query"][sl], dtype=np.float32),
                "keys": np.ascontiguousarray(inputs["keys"][sl], dtype=np.float32),
                "mask": np.ascontiguousarray(inputs["mask"][sl], dtype=np.int32),
                **shared,
            }
        )

    trace = bool(int(os.environ.get("BAHDANAU_TRACE", "0")))
    res = run_bass_kernel_spmd(
        nc, in_maps, core_ids=list(range(NCORES)), trace=trace
    )
    if trace and res.exec_time_ns is not None:
        print(f"HW exec time: {res.exec_time_ns} ns")
        if res.instructions_and_trace is not None:
            print(f"trace: {res.instructions_and_trace[1]}")

    context = np.concatenate([r["context"] for r in res.results], axis=0)
    weights = np.concatenate([r["weights"] for r in res.results], axis=0)
    return context, weights


if __name__ == "__main__":
    rng = np.random.default_rng(0)
    scale = 1.0 / np.sqrt(H)
    ins = {
        "query": rng.standard_normal((B, H), dtype=np.float32),
        "keys": rng.standard_normal((B, S, H), dtype=np.float32),
        "mask": rng.integers(0, 2, size=(B, S)).astype(np.int32),
        "Wa_w": rng.uniform(-scale, scale, (H, H)).astype(np.float32),
        "Wa_b": rng.uniform(-scale, scale, (H,)).astype(np.float32),
        "Ua_w": rng.uniform(-scale, scale, (H, H)).astype(np.float32),
        "Ua_b": rng.uniform(-scale, scale, (H,)).astype(np.float32),
        "Va_w": rng.uniform(-scale, scale, (1, H)).astype(np.float32),
        "Va_b": rng.uniform(-scale, scale, (1,)).astype(np.float32),
    }
    ctx, wts = kernel(**ins)
    print("context", ctx.shape, ctx.dtype, "weights", wts.shape, wts.dtype)
